# revision 6
# baseline (speedup 1.0000x reference)
"""Batched MoE (dense routing) Trainium2 kernel — PSUM-combined variant.

Reference computation (per batch row b):
    alpha = softmax(x @ Wg + bg)                      # (B, E)
    h = relu(x @ W0[e]); h = relu(h @ W1[e]); h = relu(h @ W2[e])
    h3[e] = h @ W3[e]
    y = sum_e alpha[:, e, None] * h3[e]               # (B, 128)

Shapes: B=65536, D=512, E=8, DH=128, DOUT=128.

Strategy: data-parallel shard B across 8 NeuronCores (8192 rows each);
weights replicated.  Activations live in transposed [feature, batch]
layout so every matmul streams with free dim 512 (full PE rate for
float32r) and the per-expert weight is the stationary operand.

Key restructure vs the transpose/combine variant: relu is positively
homogeneous and softmax weights are positive, so alpha can be folded
into the h2 relu eviction as a per-batch-column scale:
    yT = sum_e W3[e].T @ (alpha_e ⊙ relu(L2_e))
All 8 experts' final-layer matmuls then accumulate into a single PSUM
bank (start=e==0, stop=e==7), eliminating both the PE transposes of the
gate weights and the per-expert DVE combine of the old scheme, and
keeping L3 at full f32r rate (free dim 512 instead of 128).

The unnormalized gate weights ez = exp(logits+bg) are replicated
across the 128 partitions with DMA only: ezT [E,NB] bounces through an
8KB DRAM tile, then 8 DRAM->SBUF broadcast DMAs (partition-stride-0
reads are legal for DRAM sources) land [128,NB] replicas a full chunk
ahead of use.  Softmax normalization is deferred to the host: the
device accumulates  yT = sum_e ez_e * h3_e  and ships the raw ez rows
(128KB) out; the host divides by zsum = sum_e ez (the sum of the exact
same bf16 values the device multiplied by) and transposes yT back.
All matmul operands are bf16 (weight loads hide behind the 213ns
N=512 stream; fp32r weight loads do not), keeping rel err ~5e-3 vs
the 2e-2 gate.  Expert stages are software-pipelined with 2-stage
producer->consumer leads (L0@e, L1@e+2, L2@e+4, L3@e+6) so ACT/DVE
eviction bursts never stall the in-order PE queue; h1 relu evictions
alternate between ACT and DVE to balance the two eviction engines.
"""

import numpy as np

import concourse.tile as tile
from concourse import bacc, mybir
from concourse.bass_utils import run_bass_kernel_spmd

B, D, E, DH, DOUT = 65536, 512, 8, 128, 128
N_CORES = 8
B_LOCAL = B // N_CORES          # 8192
NB = 512                        # batch rows per chunk
CHUNKS = B_LOCAL // NB          # 16
DK = D // 128                   # 4 k-tiles over the input dim
P = 128

F32 = mybir.dt.float32
F32R = mybir.dt.float32r
BF16 = mybir.dt.bfloat16

# per-group matmul dtype: "f32r" or "bf16"
DT_X = "bf16"    # xts + gate weights + W0
DT_MID = "bf16"  # W1/W2 + h0/h1
DT_L3 = "bf16"   # W3 + h2
DT_A = "bf16"    # alphaT + onehots (the alpha-broadcast matmul)


def _sb(g):
    return BF16 if g == "bf16" else F32R


def _dram(g):
    return BF16 if g == "bf16" else F32


def _np_dt(g):
    import ml_dtypes
    return ml_dtypes.bfloat16 if g == "bf16" else np.float32

_CACHE = {}


def _build():
    if "nc" in _CACHE:
        return _CACHE["nc"]

    nc = bacc.Bacc("TRN2", target_bir_lowering=False, debug=False,
                   num_devices=N_CORES)

    xt_ap = nc.dram_tensor("xt", [D, B_LOCAL], _dram(DT_X), kind="ExternalInput").ap()
    w0_ap = nc.dram_tensor("w0", [P, E, DK, DH], _dram(DT_X), kind="ExternalInput").ap()
    w1_ap = nc.dram_tensor("w1", [P, E, DH], _dram(DT_MID), kind="ExternalInput").ap()
    w2_ap = nc.dram_tensor("w2", [P, E, DH], _dram(DT_MID), kind="ExternalInput").ap()
    w3_ap = nc.dram_tensor("w3", [P, E, DOUT], _dram(DT_L3), kind="ExternalInput").ap()
    wg_ap = nc.dram_tensor("wg", [P, DK, E], _dram(DT_X), kind="ExternalInput").ap()
    bg_ap = nc.dram_tensor("bg", [E, 1], F32, kind="ExternalInput").ap()
    ez_ap = nc.dram_tensor("ez", [CHUNKS, E, NB], _dram(DT_A),
                           kind="ExternalOutput").ap()
    yt_ap = nc.dram_tensor("yt", [DOUT, B_LOCAL], F32,
                           kind="ExternalOutput").ap()

    with tile.TileContext(nc) as tc:
        with (
            tc.tile_pool(name="weights", bufs=1) as wpool,
            tc.tile_pool(name="xt", bufs=4) as xpool,
            tc.tile_pool(name="h", bufs=4) as hpool,
            tc.tile_pool(name="soft", bufs=2) as spool,
            tc.tile_pool(name="ezrep", bufs=2) as epool,
            tc.tile_pool(name="az", bufs=2, space="DRAM") as azpool,
            tc.tile_pool(name="yt", bufs=2) as ypool,
            tc.tile_pool(name="ph0", bufs=3, space="PSUM") as ph0pool,
            tc.tile_pool(name="ph12", bufs=3, space="PSUM") as ph12pool,
            tc.tile_pool(name="py", bufs=2, space="PSUM") as pypool,
        ):
            def load_xts(c, split=False):
                xts = xpool.tile([P, DK, NB], _sb(DT_X), tag="xts", name=f"xts_{c}")
                if split:
                    # per-k-tile DMAs: the chunk-0 gate/L0 start sooner
                    for dk in range(DK):
                        nc.sync.dma_start(
                            xts[:, dk, :],
                            xt_ap[dk * P:(dk + 1) * P, c * NB:(c + 1) * NB]
                            .bitcast(_sb(DT_X)))
                else:
                    nc.sync.dma_start(
                        xts[:],
                        xt_ap[:, c * NB:(c + 1) * NB]
                        .rearrange("(dk p) b -> p dk b", p=P).bitcast(_sb(DT_X)),
                    )
                return xts

            # gate weights first (tiny), then the first x chunk, so the
            # first gate matmul can issue as soon as x's dk0 slice lands
            wg_sb = wpool.tile([P, DK, E], _sb(DT_X))
            nc.sync.dma_start(wg_sb[:], wg_ap.bitcast(_sb(DT_X)))
            xts_next = load_xts(0, split=True)
            bg_sb = wpool.tile([E, 1], F32)
            nc.sync.dma_start(bg_sb[:], bg_ap)

            w0_sb = []
            for e in range(E):
                w0e = wpool.tile([P, DK, DH], _sb(DT_X), name=f"w0_{e}")
                nc.sync.dma_start(w0e[:], w0_ap[:, e].bitcast(_sb(DT_X)))
                w0_sb.append(w0e)
            w1_sb = wpool.tile([P, E, DH], _sb(DT_MID))
            nc.sync.dma_start(w1_sb[:], w1_ap.bitcast(_sb(DT_MID)))
            w2_sb = wpool.tile([P, E, DH], _sb(DT_MID))
            nc.sync.dma_start(w2_sb[:], w2_ap.bitcast(_sb(DT_MID)))
            w3_sb = wpool.tile([P, E, DOUT], _sb(DT_L3))
            nc.sync.dma_start(w3_sb[:], w3_ap.bitcast(_sb(DT_L3)))

            def emit_gate(xts_for, idx):
                # gate: logitsT = Wg.T @ xT -> [E, b]; exp(z+bg) on ACT;
                # normalize in the tiny [E, NB] layout (gpsimd cross-
                # partition sum + DVE reciprocal/mult)
                pgt = ph12pool.tile([E, NB], F32, tag="ph12", name=f"pgt_{idx}")
                for dk in range(DK):
                    nc.tensor.matmul(pgt[:], wg_sb[:, dk, :],
                                     xts_for[:, dk, :],
                                     start=(dk == 0), stop=(dk == DK - 1))
                # unnormalized weights: ez = exp(z+bg).  Normalization is
                # deferred to the host (y = yt/zsum), so no reciprocal or
                # cross-partition reduce runs on the device.
                ezT = spool.tile([E, NB], _sb(DT_A), tag="ezT",
                                 name=f"ezT_{idx}")
                nc.scalar.activation(ezT[:], pgt[:],
                                     mybir.ActivationFunctionType.Exp,
                                     bias=bg_sb[:, 0:1])
                # ship the raw ez rows to the host, which computes the
                # softmax denominator itself (sum of the exact same bf16
                # values the device multiplies by)
                # replicate each expert's ez row across all 128
                # partitions: bounce through a DRAM tile (8KB), then
                # DRAM->SBUF broadcast DMAs (partition-stride-0 reads are
                # legal for DRAM sources).  All on idle DMA queues and
                # issued a full chunk ahead, so the latency is hidden.
                az = azpool.tile([E, NB], _sb(DT_A), tag="az",
                                 name=f"az_{idx}")
                nc.sync.dma_start(az[:], ezT[:])
                ezrep = epool.tile([P, E, NB], _sb(DT_A), tag="ezrep",
                                   name=f"ezrep_{idx}")
                for e in range(E):
                    nc.sync.dma_start(
                        ezrep[:, e, :],
                        az[e:e + 1, :].to_broadcast([P, NB]))
                nc.sync.dma_start(ez_ap[idx], ezT[:])
                return ezrep

            ezrep = emit_gate(xts_next, 0)

            for c in range(CHUNKS):
                xts = xts_next
                if c + 1 < CHUNKS:
                    xts_next = load_xts(c + 1)

                # ---- software-pipelined expert stages ----
                h0t, h1t, h2t = {}, {}, {}
                pyT = pypool.tile([P, NB], F32, tag="pyT", name=f"pyT_{c}")

                # producer->consumer lead (stages): deep in steady state,
                # shallow for the last chunk to shorten the drain
                d1, d2, d3 = 2, 4, 6
                for t in range(E + d3):
                    if t == 5 and c + 1 < CHUNKS:
                        # hoist next chunk's gate into this chunk's pipeline
                        ezrep_next = emit_gate(xts_next, c + 1)
                    if t < E:                      # L0(e=t)
                        e = t
                        ph0 = ph0pool.tile([P, NB], F32, tag="ph0")
                        for dk in range(DK):
                            nc.tensor.matmul(
                                ph0[:], w0_sb[e][:, dk, :], xts[:, dk, :],
                                start=(dk == 0), stop=(dk == DK - 1))
                        h0t[e] = hpool.tile([P, NB], _sb(DT_MID), tag="h0",
                                            name=f"h0_{c}_{e}")
                        nc.scalar.activation(
                            h0t[e][:], ph0[:],
                            mybir.ActivationFunctionType.Relu)
                    if d1 <= t <= E + d1 - 1:      # L1
                        e = t - d1
                        ph1 = ph12pool.tile([P, NB], F32, tag="ph12")
                        nc.tensor.matmul(ph1[:], w1_sb[:, e, :], h0t[e][:],
                                         start=True, stop=True)
                        h1t[e] = hpool.tile([P, NB], _sb(DT_MID), tag="h1",
                                            name=f"h1_{c}_{e}")
                        if e % 2 == 0:
                            nc.scalar.activation(
                                h1t[e][:], ph1[:],
                                mybir.ActivationFunctionType.Relu)
                        else:
                            nc.vector.tensor_scalar_max(
                                h1t[e][:], ph1[:], 0.0)
                        del h0t[e]
                    if d2 <= t <= E + d2 - 1:      # L2
                        e = t - d2
                        ph2 = ph12pool.tile([P, NB], F32, tag="ph12")
                        nc.tensor.matmul(ph2[:], w2_sb[:, e, :], h1t[e][:],
                                         start=True, stop=True)
                        # fused relu + alpha scale: h2s = max(ph2,0) * alpha
                        h2t[e] = hpool.tile([P, NB], _sb(DT_L3), tag="h2",
                                            name=f"h2_{c}_{e}")
                        nc.vector.scalar_tensor_tensor(
                            h2t[e][:], ph2[:], 0.0, ezrep[:, e, :],
                            mybir.AluOpType.max, mybir.AluOpType.mult)
                        del h1t[e]
                    if d3 <= t <= E + d3 - 1:      # L3, PSUM-combined
                        e = t - d3
                        nc.tensor.matmul(pyT[:], w3_sb[:, e, :], h2t[e][:],
                                         start=(e == 0), stop=(e == E - 1))
                        del h2t[e]

                yt_sb = ypool.tile([P, NB], F32, tag="yt", name=f"yt_{c}")
                nc.vector.tensor_copy(yt_sb[:], pyT[:])
                nc.sync.dma_start(yt_ap[:, c * NB:(c + 1) * NB], yt_sb[:])
                if c + 1 < CHUNKS:
                    ezrep = ezrep_next

    nc.compile()
    _CACHE["nc"] = nc
    return nc


def _prep_inputs(x, Wg, bg, W0, W1, W2, W3):
    x = np.ascontiguousarray(np.asarray(x, dtype=np.float32))
    Wg = np.asarray(Wg, dtype=np.float32)
    bg = np.asarray(bg, dtype=np.float32)
    W0 = np.asarray(W0, dtype=np.float32)
    W1 = np.asarray(W1, dtype=np.float32)
    W2 = np.asarray(W2, dtype=np.float32)
    W3 = np.asarray(W3, dtype=np.float32)
    assert x.shape == (B, D)

    xt = np.ascontiguousarray(x.T.astype(_np_dt(DT_X)))            # [D, B]
    w0h = np.ascontiguousarray(
        W0.reshape(E, DK, P, DH).transpose(2, 0, 1, 3).astype(_np_dt(DT_X)))
    w1h = np.ascontiguousarray(W1.transpose(1, 0, 2).astype(_np_dt(DT_MID)))
    w2h = np.ascontiguousarray(W2.transpose(1, 0, 2).astype(_np_dt(DT_MID)))
    w3h = np.ascontiguousarray(W3.transpose(1, 0, 2).astype(_np_dt(DT_L3)))
    wgh = np.ascontiguousarray(
        Wg.reshape(DK, P, E).transpose(1, 0, 2).astype(_np_dt(DT_X)))
    bgh = np.ascontiguousarray(bg.reshape(E, 1))

    in_maps = []
    for core in range(N_CORES):
        sl = slice(core * B_LOCAL, (core + 1) * B_LOCAL)
        in_maps.append({
            "xt": np.ascontiguousarray(xt[:, sl]),
            "w0": w0h, "w1": w1h, "w2": w2h, "w3": w3h,
            "wg": wgh, "bg": bgh,
        })
    return in_maps


def _run(inputs, trace=False, **kwargs):
    nc = _build()
    in_maps = _prep_inputs(**inputs)
    res = run_bass_kernel_spmd(nc, in_maps, core_ids=list(range(N_CORES)),
                               trace=trace, **kwargs)
    outs = []
    for i in range(N_CORES):
        ez = np.asarray(res.results[i]["ez"], dtype=np.float32)
        zsum = ez.sum(axis=1).reshape(-1)          # [CHUNKS*NB] = [B_LOCAL]
        outs.append(np.ascontiguousarray(
            (res.results[i]["yt"] / zsum[None, :]).T))
    y = np.concatenate(outs, axis=0)
    return y, res


def kernel(**inputs):
    y, _ = _run(inputs)
    return y


# revision 7
# speedup vs baseline: 1.0084x; 1.0084x over previous
"""Batched MoE (dense routing) Trainium2 kernel — PSUM-combined variant.

Reference computation (per batch row b):
    alpha = softmax(x @ Wg + bg)                      # (B, E)
    h = relu(x @ W0[e]); h = relu(h @ W1[e]); h = relu(h @ W2[e])
    h3[e] = h @ W3[e]
    y = sum_e alpha[:, e, None] * h3[e]               # (B, 128)

Shapes: B=65536, D=512, E=8, DH=128, DOUT=128.

Strategy: data-parallel shard B across 8 NeuronCores (8192 rows each);
weights replicated.  Activations live in transposed [feature, batch]
layout so every matmul streams with free dim 512 (full PE rate for
float32r) and the per-expert weight is the stationary operand.

Key restructure vs the transpose/combine variant: relu is positively
homogeneous and softmax weights are positive, so alpha can be folded
into the h2 relu eviction as a per-batch-column scale:
    yT = sum_e W3[e].T @ (alpha_e ⊙ relu(L2_e))
All 8 experts' final-layer matmuls then accumulate into a single PSUM
bank (start=e==0, stop=e==7), eliminating both the PE transposes of the
gate weights and the per-expert DVE combine of the old scheme, and
keeping L3 at full f32r rate (free dim 512 instead of 128).

The unnormalized gate weights ez = exp(logits+bg) are replicated
across the 128 partitions with DMA only: ezT [E,NB] bounces through an
8KB DRAM tile, then 8 DRAM->SBUF broadcast DMAs (partition-stride-0
reads are legal for DRAM sources) land [128,NB] replicas a full chunk
ahead of use.  Softmax normalization is deferred to the host: the
device accumulates  yT = sum_e ez_e * h3_e  and ships the raw ez rows
(128KB) out; the host divides by zsum = sum_e ez (the sum of the exact
same bf16 values the device multiplied by) and transposes yT back.
All matmul operands are bf16 (weight loads hide behind the 213ns
N=512 stream; fp32r weight loads do not), keeping rel err ~5e-3 vs
the 2e-2 gate.  Expert stages are software-pipelined with 2-stage
producer->consumer leads (L0@e, L1@e+2, L2@e+4, L3@e+6) so ACT/DVE
eviction bursts never stall the in-order PE queue; h1 relu evictions
alternate between ACT and DVE to balance the two eviction engines.
"""

import numpy as np

import concourse.tile as tile
from concourse import bacc, mybir
from concourse.bass_utils import run_bass_kernel_spmd

B, D, E, DH, DOUT = 65536, 512, 8, 128, 128
N_CORES = 8
B_LOCAL = B // N_CORES          # 8192
NB = 512                        # batch rows per chunk
CHUNKS = B_LOCAL // NB          # 16
DK = D // 128                   # 4 k-tiles over the input dim
P = 128

F32 = mybir.dt.float32
F32R = mybir.dt.float32r
BF16 = mybir.dt.bfloat16

# per-group matmul dtype: "f32r" or "bf16"
DT_X = "bf16"    # xts + gate weights + W0
DT_MID = "bf16"  # W1/W2 + h0/h1
DT_L3 = "bf16"   # W3 + h2
DT_A = "bf16"    # alphaT + onehots (the alpha-broadcast matmul)


def _sb(g):
    return BF16 if g == "bf16" else F32R


def _dram(g):
    return BF16 if g == "bf16" else F32


def _np_dt(g):
    import ml_dtypes
    return ml_dtypes.bfloat16 if g == "bf16" else np.float32

_CACHE = {}


def _build():
    if "nc" in _CACHE:
        return _CACHE["nc"]

    nc = bacc.Bacc("TRN2", target_bir_lowering=False, debug=False,
                   num_devices=N_CORES)

    xt_ap = nc.dram_tensor("xt", [D, B_LOCAL], _dram(DT_X), kind="ExternalInput").ap()
    w0_ap = nc.dram_tensor("w0", [P, E, DK, DH], _dram(DT_X), kind="ExternalInput").ap()
    w1_ap = nc.dram_tensor("w1", [P, E, DH], _dram(DT_MID), kind="ExternalInput").ap()
    w2_ap = nc.dram_tensor("w2", [P, E, DH], _dram(DT_MID), kind="ExternalInput").ap()
    w3_ap = nc.dram_tensor("w3", [P, E, DOUT], _dram(DT_L3), kind="ExternalInput").ap()
    wg_ap = nc.dram_tensor("wg", [P, DK, E], _dram(DT_X), kind="ExternalInput").ap()
    bg_ap = nc.dram_tensor("bg", [E, 1], F32, kind="ExternalInput").ap()
    ez_ap = nc.dram_tensor("ez", [CHUNKS, E, NB], _dram(DT_A),
                           kind="ExternalOutput").ap()
    yt_ap = nc.dram_tensor("yt", [DOUT, B_LOCAL], F32,
                           kind="ExternalOutput").ap()

    with tile.TileContext(nc) as tc:
        with (
            tc.tile_pool(name="weights", bufs=1) as wpool,
            tc.tile_pool(name="xt", bufs=4) as xpool,
            tc.tile_pool(name="h", bufs=4) as hpool,
            tc.tile_pool(name="soft", bufs=2) as spool,
            tc.tile_pool(name="ezrep", bufs=2) as epool,
            tc.tile_pool(name="az", bufs=2, space="DRAM") as azpool,
            tc.tile_pool(name="yt", bufs=2) as ypool,
            tc.tile_pool(name="ph0", bufs=2, space="PSUM") as ph0pool,
            tc.tile_pool(name="ph12", bufs=3, space="PSUM") as ph12pool,
            tc.tile_pool(name="py", bufs=2, space="PSUM") as pypool,
            tc.tile_pool(name="pgate", bufs=1, space="PSUM") as pgpool,
        ):
            def load_xts(c, split=False):
                xts = xpool.tile([P, DK, NB], _sb(DT_X), tag="xts", name=f"xts_{c}")
                if split:
                    # per-k-tile DMAs: the chunk-0 gate/L0 start sooner
                    for dk in range(DK):
                        nc.sync.dma_start(
                            xts[:, dk, :],
                            xt_ap[dk * P:(dk + 1) * P, c * NB:(c + 1) * NB]
                            .bitcast(_sb(DT_X)))
                else:
                    nc.sync.dma_start(
                        xts[:],
                        xt_ap[:, c * NB:(c + 1) * NB]
                        .rearrange("(dk p) b -> p dk b", p=P).bitcast(_sb(DT_X)),
                    )
                return xts

            # gate weights first (tiny), then the first x chunk, so the
            # first gate matmul can issue as soon as x's dk0 slice lands
            wg_sb = wpool.tile([P, DK, E], _sb(DT_X))
            nc.sync.dma_start(wg_sb[:], wg_ap.bitcast(_sb(DT_X)))
            xts_next = load_xts(0, split=True)
            bg_sb = wpool.tile([E, 1], F32)
            nc.sync.dma_start(bg_sb[:], bg_ap)

            w0_sb = []
            for e in range(E):
                w0e = wpool.tile([P, DK, DH], _sb(DT_X), name=f"w0_{e}")
                nc.sync.dma_start(w0e[:], w0_ap[:, e].bitcast(_sb(DT_X)))
                w0_sb.append(w0e)
            w1_sb = wpool.tile([P, E, DH], _sb(DT_MID))
            nc.sync.dma_start(w1_sb[:], w1_ap.bitcast(_sb(DT_MID)))
            w2_sb = wpool.tile([P, E, DH], _sb(DT_MID))
            nc.sync.dma_start(w2_sb[:], w2_ap.bitcast(_sb(DT_MID)))
            w3_sb = wpool.tile([P, E, DOUT], _sb(DT_L3))
            nc.sync.dma_start(w3_sb[:], w3_ap.bitcast(_sb(DT_L3)))

            def emit_gate(xts_for, idx):
                # gate: logitsT = Wg.T @ xT -> [E, b]; exp(z+bg) on ACT;
                # normalize in the tiny [E, NB] layout (gpsimd cross-
                # partition sum + DVE reciprocal/mult)
                pgt = pgpool.tile([E, NB], F32, tag="pgt", name=f"pgt_{idx}")
                for dk in range(DK):
                    nc.tensor.matmul(pgt[:], wg_sb[:, dk, :],
                                     xts_for[:, dk, :],
                                     start=(dk == 0), stop=(dk == DK - 1))
                # unnormalized weights: ez = exp(z+bg).  Normalization is
                # deferred to the host (y = yt/zsum), so no reciprocal or
                # cross-partition reduce runs on the device.
                ezT = spool.tile([E, NB], _sb(DT_A), tag="ezT",
                                 name=f"ezT_{idx}")
                nc.scalar.activation(ezT[:], pgt[:],
                                     mybir.ActivationFunctionType.Exp,
                                     bias=bg_sb[:, 0:1])
                # ship the raw ez rows to the host, which computes the
                # softmax denominator itself (sum of the exact same bf16
                # values the device multiplies by)
                # replicate each expert's ez row across all 128
                # partitions: bounce through a DRAM tile (8KB), then
                # DRAM->SBUF broadcast DMAs (partition-stride-0 reads are
                # legal for DRAM sources).  All on idle DMA queues and
                # issued a full chunk ahead, so the latency is hidden.
                az = azpool.tile([E, NB], _sb(DT_A), tag="az",
                                 name=f"az_{idx}")
                nc.sync.dma_start(az[:], ezT[:])
                ezrep = epool.tile([P, E, NB], _sb(DT_A), tag="ezrep",
                                   name=f"ezrep_{idx}")
                for e in range(E):
                    nc.sync.dma_start(
                        ezrep[:, e, :],
                        az[e:e + 1, :].to_broadcast([P, NB]))
                nc.sync.dma_start(ez_ap[idx], ezT[:])
                return ezrep

            ezrep = emit_gate(xts_next, 0)

            for c in range(CHUNKS):
                xts = xts_next
                if c + 1 < CHUNKS:
                    xts_next = load_xts(c + 1)

                # ---- software-pipelined expert stages ----
                h0t, h1t, h2t = {}, {}, {}
                pyT = pypool.tile([P, NB], F32, tag="pyT", name=f"pyT_{c}")

                # producer->consumer lead (stages): deep in steady state,
                # shallow for the last chunk to shorten the drain
                d1, d2, d3 = 2, 4, 6
                for t in range(E + d3):
                    if t == 5 and c + 1 < CHUNKS:
                        # hoist next chunk's gate into this chunk's pipeline
                        ezrep_next = emit_gate(xts_next, c + 1)
                    if t < E:                      # L0(e=t)
                        e = t
                        ph0 = ph0pool.tile([P, NB], F32, tag="ph0")
                        for dk in range(DK):
                            nc.tensor.matmul(
                                ph0[:], w0_sb[e][:, dk, :], xts[:, dk, :],
                                start=(dk == 0), stop=(dk == DK - 1))
                        h0t[e] = hpool.tile([P, NB], _sb(DT_MID), tag="h0",
                                            name=f"h0_{c}_{e}")
                        nc.scalar.activation(
                            h0t[e][:], ph0[:],
                            mybir.ActivationFunctionType.Relu)
                    if d1 <= t <= E + d1 - 1:      # L1
                        e = t - d1
                        ph1 = ph12pool.tile([P, NB], F32, tag="ph12")
                        nc.tensor.matmul(ph1[:], w1_sb[:, e, :], h0t[e][:],
                                         start=True, stop=True)
                        h1t[e] = hpool.tile([P, NB], _sb(DT_MID), tag="h1",
                                            name=f"h1_{c}_{e}")
                        if e % 2 == 0:
                            nc.scalar.activation(
                                h1t[e][:], ph1[:],
                                mybir.ActivationFunctionType.Relu)
                        else:
                            nc.vector.tensor_scalar_max(
                                h1t[e][:], ph1[:], 0.0)
                        del h0t[e]
                    if d2 <= t <= E + d2 - 1:      # L2
                        e = t - d2
                        ph2 = ph12pool.tile([P, NB], F32, tag="ph12")
                        nc.tensor.matmul(ph2[:], w2_sb[:, e, :], h1t[e][:],
                                         start=True, stop=True)
                        # fused relu + alpha scale: h2s = max(ph2,0) * alpha
                        h2t[e] = hpool.tile([P, NB], _sb(DT_L3), tag="h2",
                                            name=f"h2_{c}_{e}")
                        nc.vector.scalar_tensor_tensor(
                            h2t[e][:], ph2[:], 0.0, ezrep[:, e, :],
                            mybir.AluOpType.max, mybir.AluOpType.mult)
                        del h1t[e]
                    if d3 <= t <= E + d3 - 1:      # L3, PSUM-combined
                        e = t - d3
                        nc.tensor.matmul(pyT[:], w3_sb[:, e, :], h2t[e][:],
                                         start=(e == 0), stop=(e == E - 1))
                        del h2t[e]

                yt_sb = ypool.tile([P, NB], F32, tag="yt", name=f"yt_{c}")
                nc.vector.tensor_copy(yt_sb[:], pyT[:])
                nc.sync.dma_start(yt_ap[:, c * NB:(c + 1) * NB], yt_sb[:])
                if c + 1 < CHUNKS:
                    ezrep = ezrep_next

    nc.compile()
    _CACHE["nc"] = nc
    return nc


def _prep_inputs(x, Wg, bg, W0, W1, W2, W3):
    x = np.ascontiguousarray(np.asarray(x, dtype=np.float32))
    Wg = np.asarray(Wg, dtype=np.float32)
    bg = np.asarray(bg, dtype=np.float32)
    W0 = np.asarray(W0, dtype=np.float32)
    W1 = np.asarray(W1, dtype=np.float32)
    W2 = np.asarray(W2, dtype=np.float32)
    W3 = np.asarray(W3, dtype=np.float32)
    assert x.shape == (B, D)

    xt = np.ascontiguousarray(x.T.astype(_np_dt(DT_X)))            # [D, B]
    w0h = np.ascontiguousarray(
        W0.reshape(E, DK, P, DH).transpose(2, 0, 1, 3).astype(_np_dt(DT_X)))
    w1h = np.ascontiguousarray(W1.transpose(1, 0, 2).astype(_np_dt(DT_MID)))
    w2h = np.ascontiguousarray(W2.transpose(1, 0, 2).astype(_np_dt(DT_MID)))
    w3h = np.ascontiguousarray(W3.transpose(1, 0, 2).astype(_np_dt(DT_L3)))
    wgh = np.ascontiguousarray(
        Wg.reshape(DK, P, E).transpose(1, 0, 2).astype(_np_dt(DT_X)))
    bgh = np.ascontiguousarray(bg.reshape(E, 1))

    in_maps = []
    for core in range(N_CORES):
        sl = slice(core * B_LOCAL, (core + 1) * B_LOCAL)
        in_maps.append({
            "xt": np.ascontiguousarray(xt[:, sl]),
            "w0": w0h, "w1": w1h, "w2": w2h, "w3": w3h,
            "wg": wgh, "bg": bgh,
        })
    return in_maps


def _run(inputs, trace=False, **kwargs):
    nc = _build()
    in_maps = _prep_inputs(**inputs)
    res = run_bass_kernel_spmd(nc, in_maps, core_ids=list(range(N_CORES)),
                               trace=trace, **kwargs)
    outs = []
    for i in range(N_CORES):
        ez = np.asarray(res.results[i]["ez"], dtype=np.float32)
        zsum = ez.sum(axis=1).reshape(-1)          # [CHUNKS*NB] = [B_LOCAL]
        outs.append(np.ascontiguousarray(
            (res.results[i]["yt"] / zsum[None, :]).T))
    y = np.concatenate(outs, axis=0)
    return y, res


def kernel(**inputs):
    y, _ = _run(inputs)
    return y


# revision 8
# speedup vs baseline: 1.0145x; 1.0060x over previous
"""Batched MoE (dense routing) Trainium2 kernel — PSUM-combined variant.

Reference computation (per batch row b):
    alpha = softmax(x @ Wg + bg)                      # (B, E)
    h = relu(x @ W0[e]); h = relu(h @ W1[e]); h = relu(h @ W2[e])
    h3[e] = h @ W3[e]
    y = sum_e alpha[:, e, None] * h3[e]               # (B, 128)

Shapes: B=65536, D=512, E=8, DH=128, DOUT=128.

Strategy: data-parallel shard B across 8 NeuronCores (8192 rows each);
weights replicated.  Activations live in transposed [feature, batch]
layout so every matmul streams with free dim 512 (full PE rate for
float32r) and the per-expert weight is the stationary operand.

Key restructure vs the transpose/combine variant: relu is positively
homogeneous and softmax weights are positive, so alpha can be folded
into the h2 relu eviction as a per-batch-column scale:
    yT = sum_e W3[e].T @ (alpha_e ⊙ relu(L2_e))
All 8 experts' final-layer matmuls then accumulate into a single PSUM
bank (start=e==0, stop=e==7), eliminating both the PE transposes of the
gate weights and the per-expert DVE combine of the old scheme, and
keeping L3 at full f32r rate (free dim 512 instead of 128).

The unnormalized gate weights ez = exp(logits+bg) are replicated
across the 128 partitions with DMA only: ezT [E,NB] bounces through an
8KB DRAM tile, then 8 DRAM->SBUF broadcast DMAs (partition-stride-0
reads are legal for DRAM sources) land [128,NB] replicas a full chunk
ahead of use.  Softmax normalization is deferred to the host: the
device accumulates  yT = sum_e ez_e * h3_e  and ships the raw ez rows
(128KB) out; the host divides by zsum = sum_e ez (the sum of the exact
same bf16 values the device multiplied by) and transposes yT back.
All matmul operands are bf16 (weight loads hide behind the 213ns
N=512 stream; fp32r weight loads do not), keeping rel err ~5e-3 vs
the 2e-2 gate.  Expert stages are software-pipelined with 2-stage
producer->consumer leads (L0@e, L1@e+2, L2@e+4, L3@e+6) so ACT/DVE
eviction bursts never stall the in-order PE queue; h1 relu evictions
alternate between ACT and DVE to balance the two eviction engines.
"""

import numpy as np

import concourse.tile as tile
from concourse import bacc, mybir
from concourse.bass_utils import run_bass_kernel_spmd

B, D, E, DH, DOUT = 65536, 512, 8, 128, 128
N_CORES = 8
B_LOCAL = B // N_CORES          # 8192
NB = 512                        # batch rows per chunk
CHUNKS = B_LOCAL // NB          # 16
DK = D // 128                   # 4 k-tiles over the input dim
P = 128

F32 = mybir.dt.float32
F32R = mybir.dt.float32r
BF16 = mybir.dt.bfloat16

# per-group matmul dtype: "f32r" or "bf16"
DT_X = "bf16"    # xts + gate weights + W0
DT_MID = "bf16"  # W1/W2 + h0/h1
DT_L3 = "bf16"   # W3 + h2
DT_A = "bf16"    # alphaT + onehots (the alpha-broadcast matmul)


def _sb(g):
    return BF16 if g == "bf16" else F32R


def _dram(g):
    return BF16 if g == "bf16" else F32


def _np_dt(g):
    import ml_dtypes
    return ml_dtypes.bfloat16 if g == "bf16" else np.float32

_CACHE = {}


def _build():
    if "nc" in _CACHE:
        return _CACHE["nc"]

    nc = bacc.Bacc("TRN2", target_bir_lowering=False, debug=False,
                   num_devices=N_CORES)

    xt_ap = nc.dram_tensor("xt", [D, B_LOCAL], _dram(DT_X), kind="ExternalInput").ap()
    w0_ap = nc.dram_tensor("w0", [P, E, DK, DH], _dram(DT_X), kind="ExternalInput").ap()
    w1_ap = nc.dram_tensor("w1", [P, E, DH], _dram(DT_MID), kind="ExternalInput").ap()
    w2_ap = nc.dram_tensor("w2", [P, E, DH], _dram(DT_MID), kind="ExternalInput").ap()
    w3_ap = nc.dram_tensor("w3", [P, E, DOUT], _dram(DT_L3), kind="ExternalInput").ap()
    wg_ap = nc.dram_tensor("wg", [P, DK, E], _dram(DT_X), kind="ExternalInput").ap()
    bg_ap = nc.dram_tensor("bg", [E, 1], F32, kind="ExternalInput").ap()
    ez_ap = nc.dram_tensor("ez", [CHUNKS, E, NB], _dram(DT_A),
                           kind="ExternalOutput").ap()
    yt_ap = nc.dram_tensor("yt", [DOUT, B_LOCAL], F32,
                           kind="ExternalOutput").ap()

    with tile.TileContext(nc) as tc:
        with (
            tc.tile_pool(name="weights", bufs=1) as wpool,
            tc.tile_pool(name="xt", bufs=4) as xpool,
            tc.tile_pool(name="h", bufs=6) as hpool,
            tc.tile_pool(name="soft", bufs=2) as spool,
            tc.tile_pool(name="ezrep", bufs=2) as epool,
            tc.tile_pool(name="az", bufs=2, space="DRAM") as azpool,
            tc.tile_pool(name="yt", bufs=2) as ypool,
            tc.tile_pool(name="ph0", bufs=2, space="PSUM") as ph0pool,
            tc.tile_pool(name="ph12", bufs=3, space="PSUM") as ph12pool,
            tc.tile_pool(name="py", bufs=2, space="PSUM") as pypool,
            tc.tile_pool(name="pgate", bufs=1, space="PSUM") as pgpool,
        ):
            def load_xts(c, split=False):
                xts = xpool.tile([P, DK, NB], _sb(DT_X), tag="xts", name=f"xts_{c}")
                if split:
                    # per-k-tile DMAs: the chunk-0 gate/L0 start sooner
                    for dk in range(DK):
                        nc.sync.dma_start(
                            xts[:, dk, :],
                            xt_ap[dk * P:(dk + 1) * P, c * NB:(c + 1) * NB]
                            .bitcast(_sb(DT_X)))
                else:
                    nc.sync.dma_start(
                        xts[:],
                        xt_ap[:, c * NB:(c + 1) * NB]
                        .rearrange("(dk p) b -> p dk b", p=P).bitcast(_sb(DT_X)),
                    )
                return xts

            # gate weights first (tiny), then the first x chunk, so the
            # first gate matmul can issue as soon as x's dk0 slice lands
            wg_sb = wpool.tile([P, DK, E], _sb(DT_X))
            nc.sync.dma_start(wg_sb[:], wg_ap.bitcast(_sb(DT_X)))
            xts_next = load_xts(0, split=True)
            bg_sb = wpool.tile([E, 1], F32)
            nc.sync.dma_start(bg_sb[:], bg_ap)

            w0_sb = []
            for e in range(E):
                w0e = wpool.tile([P, DK, DH], _sb(DT_X), name=f"w0_{e}")
                nc.sync.dma_start(w0e[:], w0_ap[:, e].bitcast(_sb(DT_X)))
                w0_sb.append(w0e)
            w1_sb = wpool.tile([P, E, DH], _sb(DT_MID))
            nc.sync.dma_start(w1_sb[:], w1_ap.bitcast(_sb(DT_MID)))
            w2_sb = wpool.tile([P, E, DH], _sb(DT_MID))
            nc.sync.dma_start(w2_sb[:], w2_ap.bitcast(_sb(DT_MID)))
            w3_sb = wpool.tile([P, E, DOUT], _sb(DT_L3))
            nc.sync.dma_start(w3_sb[:], w3_ap.bitcast(_sb(DT_L3)))

            def emit_gate(xts_for, idx):
                # gate: logitsT = Wg.T @ xT -> [E, b]; exp(z+bg) on ACT;
                # normalize in the tiny [E, NB] layout (gpsimd cross-
                # partition sum + DVE reciprocal/mult)
                pgt = pgpool.tile([E, NB], F32, tag="pgt", name=f"pgt_{idx}")
                for dk in range(DK):
                    nc.tensor.matmul(pgt[:], wg_sb[:, dk, :],
                                     xts_for[:, dk, :],
                                     start=(dk == 0), stop=(dk == DK - 1))
                # unnormalized weights: ez = exp(z+bg).  Normalization is
                # deferred to the host (y = yt/zsum), so no reciprocal or
                # cross-partition reduce runs on the device.
                ezT = spool.tile([E, NB], _sb(DT_A), tag="ezT",
                                 name=f"ezT_{idx}")
                nc.scalar.activation(ezT[:], pgt[:],
                                     mybir.ActivationFunctionType.Exp,
                                     bias=bg_sb[:, 0:1])
                # ship the raw ez rows to the host, which computes the
                # softmax denominator itself (sum of the exact same bf16
                # values the device multiplies by)
                # replicate each expert's ez row across all 128
                # partitions: bounce through a DRAM tile (8KB), then
                # DRAM->SBUF broadcast DMAs (partition-stride-0 reads are
                # legal for DRAM sources).  All on idle DMA queues and
                # issued a full chunk ahead, so the latency is hidden.
                az = azpool.tile([E, NB], _sb(DT_A), tag="az",
                                 name=f"az_{idx}")
                nc.sync.dma_start(az[:], ezT[:])
                ezrep = epool.tile([P, E, NB], _sb(DT_A), tag="ezrep",
                                   name=f"ezrep_{idx}")
                for e in range(E):
                    nc.sync.dma_start(
                        ezrep[:, e, :],
                        az[e:e + 1, :].to_broadcast([P, NB]))
                nc.sync.dma_start(ez_ap[idx], ezT[:])
                return ezrep

            ezrep = emit_gate(xts_next, 0)

            for c in range(CHUNKS):
                xts = xts_next
                if c + 1 < CHUNKS:
                    xts_next = load_xts(c + 1)

                # ---- software-pipelined expert stages ----
                h0t, h1t, h2t = {}, {}, {}
                pyT = pypool.tile([P, NB], F32, tag="pyT", name=f"pyT_{c}")

                # producer->consumer lead (stages): deep in steady state,
                # shallow for the last chunk to shorten the drain
                d1, d2, d3 = 3, 5, 7
                for t in range(E + d3):
                    if t == 5 and c + 1 < CHUNKS:
                        # hoist next chunk's gate into this chunk's pipeline
                        ezrep_next = emit_gate(xts_next, c + 1)
                    if t < E:                      # L0(e=t)
                        e = t
                        ph0 = ph0pool.tile([P, NB], F32, tag="ph0")
                        for dk in range(DK):
                            nc.tensor.matmul(
                                ph0[:], w0_sb[e][:, dk, :], xts[:, dk, :],
                                start=(dk == 0), stop=(dk == DK - 1))
                        h0t[e] = hpool.tile([P, NB], _sb(DT_MID), tag="h0",
                                            name=f"h0_{c}_{e}")
                        nc.scalar.activation(
                            h0t[e][:], ph0[:],
                            mybir.ActivationFunctionType.Relu)
                    if d1 <= t <= E + d1 - 1:      # L1
                        e = t - d1
                        ph1 = ph12pool.tile([P, NB], F32, tag="ph12")
                        nc.tensor.matmul(ph1[:], w1_sb[:, e, :], h0t[e][:],
                                         start=True, stop=True)
                        h1t[e] = hpool.tile([P, NB], _sb(DT_MID), tag="h1",
                                            name=f"h1_{c}_{e}")
                        if e % 2 == 0:
                            nc.scalar.activation(
                                h1t[e][:], ph1[:],
                                mybir.ActivationFunctionType.Relu)
                        else:
                            nc.vector.tensor_scalar_max(
                                h1t[e][:], ph1[:], 0.0)
                        del h0t[e]
                    if d2 <= t <= E + d2 - 1:      # L2
                        e = t - d2
                        ph2 = ph12pool.tile([P, NB], F32, tag="ph12")
                        nc.tensor.matmul(ph2[:], w2_sb[:, e, :], h1t[e][:],
                                         start=True, stop=True)
                        # fused relu + alpha scale: h2s = max(ph2,0) * alpha
                        h2t[e] = hpool.tile([P, NB], _sb(DT_L3), tag="h2",
                                            name=f"h2_{c}_{e}")
                        nc.vector.scalar_tensor_tensor(
                            h2t[e][:], ph2[:], 0.0, ezrep[:, e, :],
                            mybir.AluOpType.max, mybir.AluOpType.mult)
                        del h1t[e]
                    if d3 <= t <= E + d3 - 1:      # L3, PSUM-combined
                        e = t - d3
                        nc.tensor.matmul(pyT[:], w3_sb[:, e, :], h2t[e][:],
                                         start=(e == 0), stop=(e == E - 1))
                        del h2t[e]

                yt_sb = ypool.tile([P, NB], F32, tag="yt", name=f"yt_{c}")
                nc.vector.tensor_copy(yt_sb[:], pyT[:])
                nc.sync.dma_start(yt_ap[:, c * NB:(c + 1) * NB], yt_sb[:])
                if c + 1 < CHUNKS:
                    ezrep = ezrep_next

    nc.compile()
    _CACHE["nc"] = nc
    return nc


def _prep_inputs(x, Wg, bg, W0, W1, W2, W3):
    x = np.ascontiguousarray(np.asarray(x, dtype=np.float32))
    Wg = np.asarray(Wg, dtype=np.float32)
    bg = np.asarray(bg, dtype=np.float32)
    W0 = np.asarray(W0, dtype=np.float32)
    W1 = np.asarray(W1, dtype=np.float32)
    W2 = np.asarray(W2, dtype=np.float32)
    W3 = np.asarray(W3, dtype=np.float32)
    assert x.shape == (B, D)

    xt = np.ascontiguousarray(x.T.astype(_np_dt(DT_X)))            # [D, B]
    w0h = np.ascontiguousarray(
        W0.reshape(E, DK, P, DH).transpose(2, 0, 1, 3).astype(_np_dt(DT_X)))
    w1h = np.ascontiguousarray(W1.transpose(1, 0, 2).astype(_np_dt(DT_MID)))
    w2h = np.ascontiguousarray(W2.transpose(1, 0, 2).astype(_np_dt(DT_MID)))
    w3h = np.ascontiguousarray(W3.transpose(1, 0, 2).astype(_np_dt(DT_L3)))
    wgh = np.ascontiguousarray(
        Wg.reshape(DK, P, E).transpose(1, 0, 2).astype(_np_dt(DT_X)))
    bgh = np.ascontiguousarray(bg.reshape(E, 1))

    in_maps = []
    for core in range(N_CORES):
        sl = slice(core * B_LOCAL, (core + 1) * B_LOCAL)
        in_maps.append({
            "xt": np.ascontiguousarray(xt[:, sl]),
            "w0": w0h, "w1": w1h, "w2": w2h, "w3": w3h,
            "wg": wgh, "bg": bgh,
        })
    return in_maps


def _run(inputs, trace=False, **kwargs):
    nc = _build()
    in_maps = _prep_inputs(**inputs)
    res = run_bass_kernel_spmd(nc, in_maps, core_ids=list(range(N_CORES)),
                               trace=trace, **kwargs)
    outs = []
    for i in range(N_CORES):
        ez = np.asarray(res.results[i]["ez"], dtype=np.float32)
        zsum = ez.sum(axis=1).reshape(-1)          # [CHUNKS*NB] = [B_LOCAL]
        outs.append(np.ascontiguousarray(
            (res.results[i]["yt"] / zsum[None, :]).T))
    y = np.concatenate(outs, axis=0)
    return y, res


def kernel(**inputs):
    y, _ = _run(inputs)
    return y
